# revision 9
# baseline (speedup 1.0000x reference)
"""Multi-Head Latent Attention Trainium2 kernel (8-core data parallel), v2.

Sharding: pure data parallel over (batch=4) x (sequence halves=2) = 8 cores.
Each core computes full attention output for its 1024 query tokens of one
batch, using all 2048 keys/values of that batch. No collectives.

v2 changes vs v1 (584us baseline):
  - bf16 everywhere on the matmul path (x, latents, Z, V, P, proj weights):
    halves input DMA, 4x faster V matmuls (f32r N=128 runs at 4cyc/row),
    faster DVE evacuations.
  - softmax exp split across TWO engines: ACT (true exp, bf16 out) and DVE
    (Schraudolph bit-trick exp: i16 = round(s*A + B) bitcast to bf16; one
    fused tensor_scalar per tile). The multiplicative center of the
    Schraudolph sawtooth is calibrated (C=7.35) so ACT- and DVE-computed
    key-chunks of one softmax row agree in scale; residual error is a 1.8%
    rms zero-mean sawtooth which averages out in the PV contraction.
    exp was the critical path (33.5M elem/core at 1 elem/cyc/lane @1.2GHz
    = 294us on ACT alone); splitting pushes the wall to the PE (~210us).
  - V matmuls grouped 4 kt per psum bank -> 4x fewer DVE evac instructions.
  - normalize: reciprocal straight from psum denominator row, muls read y
    from psum (no [65,512] staging copy).

Per-core kernel:
  1. latent.T [128, 2048] = [w_kv_a; w_q_a] @ x.T (fused both paths, bf16)
  2. rmsnorm along partitions: sum-of-squares via ones-matmul,
     inv_rms = exp(-0.5*ln(ms+eps)) on ACT, broadcast via DRAM-bounce DMA.
  3. per head-pair (A=2hp, B=2hp+1): folded S-matrices M_h = Wq_h^T Wk_h
     give Z = [M_A|M_B]^T @ L_q; S^T = L_k^T @ Z via row-group-concurrent
     K=64 matmul pairs into a shared [128,1024] psum tile; P = exp(S*scale)
     on ACT or DVE per a 13-cycle pattern; y_aug.T accumulated over k-tiles
     (M=65 incl. ones row = softmax denominators).
  4. proj: y.T chunks as lhsT against w_proj.T, accumulate over 16 heads.

Token order per core: [own 1024 queries, other half] so the SPMD NEFF always
reads queries at offset 0 (K/V order irrelevant to softmax).
"""
import numpy as np
import ml_dtypes

import concourse.bacc as bacc
import concourse.bass as bass
import concourse.mybir as mybir
import concourse.tile as tile
from concourse.bass_utils import run_bass_kernel_spmd

F32 = mybir.dt.float32
F32R = mybir.dt.float32r
BF16 = mybir.dt.bfloat16
I16 = mybir.dt.int16
AF = mybir.ActivationFunctionType
ALU = mybir.AluOpType

B, N, C = 4, 2048, 1024
H, D, R = 16, 64, 64
NT = 2048          # kv tokens per core (full batch sequence)
NQ = 1024          # query tokens per core
EPS = 1e-6
SCALE = D ** -0.5
N_CORES = 8

# Schraudolph exp in bf16 bit-space: bf16(bits = round(x*A + B)) ~= exp(x)
A_SCH = float((128.0 / np.log(2.0)) * SCALE)
B_SCH = float(127 * 128) - 7.35
# which exp tiles go to DVE (indices mod 13); 5/13 ~= 38% of tiles
DVE_EXP_SLOTS = frozenset((0, 3, 5, 8, 10))

# bumped on every kernel revision; padded into ones2_t's shape so the HLO
# fingerprint changes and the terminal-side staged-executable cache cannot
# serve a stale NEFF for a prior revision
KERNEL_VERSION = 4


def build_nc(reps: int = 1, ablate=()):
    ablate = set(ablate)
    nc = bacc.Bacc("TRN2", target_bir_lowering=False)
    x_t = nc.dram_tensor("x_t", [C, NT], BF16, kind="ExternalInput")
    wa_t = nc.dram_tensor("wa_t", [C, 2 * R], BF16, kind="ExternalInput")
    m_t = nc.dram_tensor("m_t", [8 * 128, 128], BF16, kind="ExternalInput")
    wv_t = nc.dram_tensor("wv_t", [R, H * D], BF16, kind="ExternalInput")
    wp_t = nc.dram_tensor("wp_t", [H * D, C], BF16, kind="ExternalInput")
    ones2_t = nc.dram_tensor("ones2_t", [128, 2 + KERNEL_VERSION], F32R,
                             kind="ExternalInput")
    y_out = nc.dram_tensor("y_out", [NQ, C], F32, kind="ExternalOutput")
    out_r = y_out.rearrange("(qt p) c -> p qt c", p=128)

    exp_ctr = [0]

    with tile.TileContext(nc) as tc:
        with (
            tc.tile_pool(name="wsb", bufs=1) as wsb,
            tc.tile_pool(name="res", bufs=1) as res,
            tc.tile_pool(name="xs", bufs=4) as xs,
            tc.tile_pool(name="work", bufs=2) as work,
            tc.tile_pool(name="pts", bufs=4) as pts,
            tc.tile_pool(name="small", bufs=2) as small,
            tc.tile_pool(name="drp", bufs=2, space="DRAM") as drp,
        ):

            def bcast_ap(dram_row, n_part):
                return bass.AP(tensor=dram_row.tensor, offset=dram_row.offset,
                               ap=[[0, n_part]] + list(dram_row.ap[1:]))

            def emit_exp(pt, st):
                """P = exp(S*scale): ACT true exp or DVE Schraudolph."""
                slot = exp_ctr[0] % 13
                exp_ctr[0] += 1
                if slot in DVE_EXP_SLOTS and "noschraud" not in ablate:
                    # two ops: a DVE PSUM read must stay within one 2KB bank
                    pt_i16 = pt.bitcast(I16)
                    nc.vector.tensor_scalar(pt_i16[:, 0:512], st[:, 0:512],
                                            A_SCH, B_SCH, ALU.mult, ALU.add)
                    nc.vector.tensor_scalar(pt_i16[:, 512:1024], st[:, 512:1024],
                                            A_SCH, B_SCH, ALU.mult, ALU.add)
                else:
                    nc.scalar.activation(pt, st, AF.Exp, scale=SCALE)

            import contextlib

            def loop_ctx():
                if reps > 1:
                    return tc.For_i(0, reps, 1)
                return contextlib.nullcontext()

            with loop_ctx():
                # ---- weights ----
                wa_sb = wsb.tile([128, 8, 2 * R], BF16, tag="wa")
                for kc in range(8):
                    nc.sync.dma_start(wa_sb[:, kc, :], wa_t[kc * 128:(kc + 1) * 128, :])
                m_sb = wsb.tile([128, 8, 128], BF16, tag="m")
                for hp in range(8):
                    nc.sync.dma_start(m_sb[:, hp, :],
                                      m_t[hp * 128:(hp + 1) * 128, :])
                wv_sb = wsb.tile([128, H * D], BF16, tag="wv")
                nc.sync.dma_start(wv_sb[0:64, :], wv_t[:])
                nc.sync.dma_start(wv_sb[64:128, :], wv_t[:])
                wp_sb = wsb.tile([128, 8, C], BF16, tag="wp")
                for hp in range(8):
                    nc.sync.dma_start(wp_sb[:, hp, :], wp_t[hp * 128:(hp + 1) * 128, :])

                # ---- constants ----
                ones_bf = wsb.tile([128, 1], BF16, tag="ones")
                nc.vector.memset(ones_bf[:], 1.0)
                eps2 = wsb.tile([2, 1], F32, tag="eps")
                nc.vector.memset(eps2[:], EPS)
                ones2 = wsb.tile([128, 2], F32R, tag="ones2")
                nc.sync.dma_start(ones2[:], ones2_t[:, 0:2])

                # ---- resident tensors ----
                lat_n = res.tile([128, NT], BF16, tag="lat_n")
                ybuf = res.tile([128, 8, NQ], BF16, tag="ybuf")

                # ---- phase 0: fused latents + rmsnorm ----
                with tc.tile_pool(name="ps0", bufs=2, space="PSUM") as ps0:
                    lat_ps = ps0.tile([128, NT], F32, tag="lat", bufs=1)
                    # stream by token chunk: each [1024C, 512tok] block of
                    # x arrives as 8 sub-DMAs; latent accumulation and
                    # rmsnorm for chunk t4 overlap chunk t4+1's DMA
                    for t4 in range(4):
                        xt = xs.tile([128, 8, 512], BF16, tag="x")
                        for kc in range(8):
                            nc.sync.dma_start(
                                xt[:, kc, :],
                                x_t[kc * 128:(kc + 1) * 128,
                                    t4 * 512:(t4 + 1) * 512])
                        for kc in range(8):
                            nc.tensor.matmul(
                                lat_ps[:, t4 * 512:(t4 + 1) * 512],
                                wa_sb[:, kc, :],
                                xt[:, kc, :],
                                start=(kc == 0), stop=(kc == 7))
                    for t4 in range(4):
                        sl = bass.ts(t4, 512)
                        sq = small.tile([128, 512], F32R, tag="sq")
                        nc.scalar.activation(sq[:], lat_ps[:, sl], AF.Square)
                        ssq = ps0.tile([2, 512], F32, tag="aux")
                        nc.tensor.matmul(ssq[:], ones2[:], sq[:], start=True, stop=True)
                        lns = small.tile([2, 512], F32, tag="sq")
                        nc.scalar.activation(lns[:], ssq[:], AF.Ln,
                                             bias=eps2[:], scale=1.0 / R)
                        inv = small.tile([2, 512], F32R, tag="sq")
                        nc.scalar.activation(inv[:], lns[:], AF.Exp, scale=-0.5)
                        inv_d = drp.tile([2, 512], F32R, tag="inv_d")
                        nc.sync.dma_start(inv_d[:], inv[:])
                        bc_sb = small.tile([128, 512], F32R, tag="sq")
                        nc.sync.dma_start(bc_sb[0:64, :], bcast_ap(inv_d[0:1, :], 64))
                        nc.sync.dma_start(bc_sb[64:128, :],
                                          bcast_ap(inv_d[1:2, :], 64))
                        nc.vector.tensor_mul(lat_n[:, sl], lat_ps[:, sl], bc_sb[:])

                # ---- phase 1: head pairs ----
                if "p0only" in ablate:
                    for qt in range(4):
                        osb0 = small.tile([128, 512], F32, tag="osb")
                        nc.vector.tensor_copy(osb0[:], lat_n[:, bass.ts(qt, 512)])
                        nc.sync.dma_start(out_r[:, qt, 0:512], osb0[:])
                    nc.compile()
                    return nc
                # duplicate kv-latent at partitions 64-127 (row-group pairing)
                lat_kv2 = res.tile([128, NT], BF16, tag="lat_kv2")
                nc.sync.dma_start(lat_kv2[64:128, :], lat_n[0:64, :])

                with (
                    tc.tile_pool(name="pst", bufs=2, space="PSUM") as pst,
                    tc.tile_pool(name="psy", bufs=2, space="PSUM") as psy,
                ):
                    def kqv_tiles_and_thunks(hp):
                        """Allocate K.T/Q.T/V tiles for pair hp and return a
                        list of emission thunks (matmul+evac units)."""
                        hsl = bass.ts(hp, 128)
                        zpr = work.tile([128, NQ], BF16, tag="zpr",
                                        name=f"zpr{hp}")
                        vt = work.tile([128, 16, 130], BF16, tag="vt",
                                       name=f"vt{hp}")
                        vt2 = vt.rearrange("p k (s u) -> p k s u", s=2)
                        thunks = []

                        def z_unit(t2):
                            # Z_pair = [M_A | M_B]^T @ L_q: one M=128 matmul
                            # produces both heads' Z (rows 0-63 A, 64-127 B)
                            sl = bass.ts(t2, 512)
                            zps = pst.tile([128, 512], F32, tag="kqv")
                            nc.tensor.matmul(zps[:],
                                             m_sb[64:128, hp, :],
                                             lat_n[64:128, sl],
                                             start=True, stop=True)
                            nc.vector.tensor_copy(zpr[:, sl], zps[:])

                        def v_unit(kt0):
                            if "basev" in ablate:
                                for kt in (kt0, kt0 + 1, kt0 + 2, kt0 + 3):
                                    vps = pst.tile([128, 128], F32, tag="kqv")
                                    if kt % 2 == 0:
                                        nc.tensor.matmul(
                                            vps[:], lat_n[0:64, bass.ts(kt, 128)],
                                            wv_sb[0:64, hsl], start=True, stop=True)
                                    else:
                                        nc.tensor.matmul(
                                            vps[:], lat_kv2[64:128, bass.ts(kt, 128)],
                                            wv_sb[64:128, hsl], start=True, stop=True)
                                    nc.vector.tensor_copy(
                                        vt2[:, kt, :, 0:64],
                                        vps[:].rearrange("p (s u) -> p s u", s=2))
                                return
                            # four V k-tiles into one psum bank, all on the
                            # SAME row group so the matmuls serialize -- two
                            # row-group-concurrent matmuls draining into one
                            # single-port psum bank is a fatal HW collision.
                            # Single strided DVE evac for all four.
                            vps = pst.tile([128, 512], F32, tag="kqv")
                            for j, kt in enumerate(range(kt0, kt0 + 4)):
                                vsl = bass.ts(j, 128)
                                nc.tensor.matmul(
                                    vps[:, vsl],
                                    lat_n[0:64, bass.ts(kt, 128)],
                                    wv_sb[0:64, hsl], start=True, stop=True)
                            nc.vector.tensor_copy(
                                vt2[:, kt0:kt0 + 4, :, 0:64],
                                vps[:].rearrange("p (k s u) -> p k s u",
                                                 k=4, s=2))

                        def ones_unit():
                            nc.vector.tensor_copy(
                                vt2[:, :, :, 64:65],
                                ones_bf[:].broadcast_to([128, 16, 2, 1]))

                        for t2 in range(2):
                            thunks.append(lambda t2=t2: z_unit(t2))
                        thunks.append(ones_unit)
                        for kt0 in range(0, 16, 4):
                            thunks.append(lambda kt0=kt0: v_unit(kt0))
                        return (zpr, vt), thunks

                    NHP = 1 if "onehp" in ablate else 8
                    cur_tiles, cur_thunks = kqv_tiles_and_thunks(0)
                    for th in cur_thunks:
                        th()
                    pending = []
                    for hp in range(NHP):
                        zpr, vt = cur_tiles
                        if hp < NHP - 1:
                            cur_tiles, pending = kqv_tiles_and_thunks(hp + 1)
                        else:
                            pending = []
                        # attention: per qc a single chain; per kt one
                        # [128,1024] (A|B) psum group -> one exp; y matmuls
                        # delayed one kt (PE FIFO never blocks on exp).
                        for qc in range(2):
                            qsl = bass.ts(qc, 512)
                            ya = psy.tile([65, 512], F32, tag="y",
                                          name=f"ya{hp}_{qc}")
                            yb = psy.tile([65, 512], F32, tag="y",
                                          name=f"yb{hp}_{qc}")

                            def emit_y(kt, pt, ya=ya, yb=yb, vt=vt):
                                nc.tensor.matmul(ya[:], vt[:, kt, 0:65],
                                                 pt[:, 0:512],
                                                 start=(kt == 0), stop=(kt == 15))
                                nc.tensor.matmul(yb[:], vt[:, kt, 65:130],
                                                 pt[:, 512:1024],
                                                 start=(kt == 0), stop=(kt == 15))

                            from collections import deque
                            pipe = deque()
                            for kt in range(16):
                                ksl = bass.ts(kt, 128)
                                st = pst.tile([128, 1024], F32, tag="st")
                                nc.tensor.matmul(st[:, 0:512],
                                                 lat_n[0:64, ksl],
                                                 zpr[0:64, qsl],
                                                 start=True, stop=True)
                                nc.tensor.matmul(st[:, 512:1024],
                                                 lat_kv2[64:128, ksl],
                                                 zpr[64:128, qsl],
                                                 start=True, stop=True)
                                pt = pts.tile([128, 1024], BF16, tag="pt")
                                emit_exp(pt[:], st[:])
                                pipe.append((kt, pt))
                                if len(pipe) > 1:
                                    emit_y(*pipe.popleft())
                                if pending:
                                    pending.pop(0)()
                            while pipe:
                                emit_y(*pipe.popleft())
                            # normalize + write into ybuf
                            for half, yp in ((0, ya), (1, yb)):
                                if "basenorm" in ablate:
                                    ysb = small.tile([65, 512], F32, tag="ysb")
                                    nc.vector.tensor_copy(ysb[:], yp[:])
                                    ysrc = ysb
                                else:
                                    ysrc = yp
                                rq = small.tile([1, 512], F32R, tag="rq")
                                with nc.allow_low_precision(
                                        reason="f32r softmax denominators"):
                                    nc.vector.reciprocal(rq[:], ysrc[64:65, :])
                                rq_d = drp.tile([1, 512], F32R, tag="rq_d")
                                nc.sync.dma_start(rq_d[:], rq[:])
                                bcy = small.tile([64, 512], F32R, tag="bcy")
                                nc.sync.dma_start(bcy[:], bcast_ap(rq_d[0:1, :], 64))
                                if half == 0:
                                    nc.vector.tensor_mul(ybuf[0:64, hp, qsl],
                                                         ysrc[0:64, :], bcy[:])
                                else:
                                    y2b = small.tile([64, 512], BF16, tag="y2b")
                                    nc.vector.tensor_mul(y2b[:], ysrc[0:64, :],
                                                         bcy[:])
                                    nc.sync.dma_start(ybuf[64:128, hp, qsl],
                                                      y2b[:])
                        for th in pending:
                            th()
                    # ---- proj ----
                    if "noproj" in ablate:
                        nc.sync.dma_start(out_r[:, :, 0:512], ybuf[:].bitcast(F32))
                        nc.compile()
                        return nc
                    for qt in range(8):
                        for cc in range(2):
                            pj = pst.tile([128, 512], F32, tag="kqv")
                            for hp2 in range(8):
                                nc.tensor.matmul(
                                    pj[:], ybuf[:, hp2, bass.ts(qt, 128)],
                                    wp_sb[:, hp2, bass.ts(cc, 512)],
                                    start=(hp2 == 0), stop=(hp2 == 7))
                            osb = small.tile([128, 512], F32, tag="osb")
                            nc.vector.tensor_copy(osb[:], pj[:])
                            nc.sync.dma_start(out_r[:, qt, bass.ts(cc, 512)],
                                              osb[:])
    nc.compile()
    return nc


BF = ml_dtypes.bfloat16


def prep_inputs(x, w_kv_a, w_kv_b, w_q_a, w_q_b, w_proj, kv_norm_w, q_norm_w):
    """Host-side sharding/layout prep. Returns per-core input maps."""
    x = np.asarray(x, dtype=np.float32)
    w_kv_b = np.asarray(w_kv_b, dtype=np.float32) * np.asarray(kv_norm_w, np.float32)[None, :]
    w_q_b = np.asarray(w_q_b, dtype=np.float32) * np.asarray(q_norm_w, np.float32)[None, :]
    wa_t = np.ascontiguousarray(
        np.concatenate([np.asarray(w_kv_a, np.float32),
                        np.asarray(w_q_a, np.float32)], axis=0).T).astype(BF)
    kvb = w_kv_b.reshape(H, 2, D, R)
    wv_t = np.ascontiguousarray(
        kvb[:, 1].transpose(2, 0, 1).reshape(R, H * D)).astype(BF)
    # per-head folded S-matrix M_h = Wq_h^T @ Wk_h  [R, R]
    m_t = np.zeros((8 * 128, 128), np.float32)
    for hp_ in range(8):
        wq_a_h = w_q_b[(2 * hp_) * D:(2 * hp_ + 1) * D, :]        # [D, R]
        wq_b_h = w_q_b[(2 * hp_ + 1) * D:(2 * hp_ + 2) * D, :]
        wk_a_h = kvb[2 * hp_, 0]                                  # [D, R]
        wk_b_h = kvb[2 * hp_ + 1, 0]
        m_t[hp_ * 128 + 64:(hp_ + 1) * 128, 0:64] = wq_a_h.T @ wk_a_h
        m_t[hp_ * 128 + 64:(hp_ + 1) * 128, 64:128] = wq_b_h.T @ wk_b_h
    m_t = m_t.astype(BF)
    wp_t = np.ascontiguousarray(np.asarray(w_proj, np.float32).T).astype(BF)

    in_maps = []
    for core in range(N_CORES):
        b, half = divmod(core, 2)
        own = x[b, half * NQ:(half + 1) * NQ]
        other = x[b, (1 - half) * NQ:(2 - half) * NQ]
        x_perm_t = np.ascontiguousarray(
            np.concatenate([own, other], axis=0).T).astype(BF)
        in_maps.append({
            "x_t": x_perm_t, "wa_t": wa_t, "m_t": m_t,
            "wv_t": wv_t, "wp_t": wp_t, "ones2_t": _ONES2,
        })
    return in_maps


def assemble_output(results):
    out = np.empty((B, N, C), dtype=np.float32)
    for core in range(N_CORES):
        b, half = divmod(core, 2)
        out[b, half * NQ:(half + 1) * NQ] = results[core]["y_out"]
    return out


_ONES2 = np.zeros((128, 2 + KERNEL_VERSION), np.float32)
_ONES2[0:64, 0] = 1.0
_ONES2[64:128, 1] = 1.0

_NC_CACHE = {}


def kernel(**inputs) -> np.ndarray:
    if 1 not in _NC_CACHE:
        _NC_CACHE[1] = build_nc(reps=1)
    nc = _NC_CACHE[1]
    in_maps = prep_inputs(**inputs)
    res = run_bass_kernel_spmd(nc, in_maps, core_ids=list(range(N_CORES)))
    return assemble_output(res.results)


# revision 17
# speedup vs baseline: 1.3599x; 1.3599x over previous
"""Multi-Head Latent Attention Trainium2 kernel (8-core data parallel), v2.

Sharding: pure data parallel over (batch=4) x (sequence halves=2) = 8 cores.
Each core computes full attention output for its 1024 query tokens of one
batch, using all 2048 keys/values of that batch. No collectives.

v2 changes vs v1 (584us baseline):
  - bf16 everywhere on the matmul path (x, latents, Z, V, P, proj weights):
    halves input DMA, 4x faster V matmuls (f32r N=128 runs at 4cyc/row),
    faster DVE evacuations.
  - softmax exp split across TWO engines: ACT (true exp, bf16 out) and DVE
    (Schraudolph bit-trick exp: i16 = round(s*A + B) bitcast to bf16; one
    fused tensor_scalar per tile). The multiplicative center of the
    Schraudolph sawtooth is calibrated (C=7.35) so ACT- and DVE-computed
    key-chunks of one softmax row agree in scale; residual error is a 1.8%
    rms zero-mean sawtooth which averages out in the PV contraction.
    exp was the critical path (33.5M elem/core at 1 elem/cyc/lane @1.2GHz
    = 294us on ACT alone); splitting pushes the wall to the PE (~210us).
  - V matmuls grouped 4 kt per psum bank -> 4x fewer DVE evac instructions.
  - normalize: reciprocal straight from psum denominator row, muls read y
    from psum (no [65,512] staging copy).

Per-core kernel:
  1. latent.T [128, 2048] = [w_kv_a; w_q_a] @ x.T (fused both paths, bf16)
  2. rmsnorm along partitions: sum-of-squares via ones-matmul,
     inv_rms = exp(-0.5*ln(ms+eps)) on ACT, broadcast via DRAM-bounce DMA.
  3. per head-pair (A=2hp, B=2hp+1): folded S-matrices M_h = Wq_h^T Wk_h
     give Z = [M_A|M_B]^T @ L_q; S^T = L_k^T @ Z via row-group-concurrent
     K=64 matmul pairs into a shared [128,1024] psum tile; P = exp(S*scale)
     on ACT or DVE per a 13-cycle pattern; y_aug.T accumulated over k-tiles
     (M=65 incl. ones row = softmax denominators).
  4. proj: y.T chunks as lhsT against w_proj.T, accumulate over 16 heads.

Token order per core: [own 1024 queries, other half] so the SPMD NEFF always
reads queries at offset 0 (K/V order irrelevant to softmax).
"""
import numpy as np
import ml_dtypes

import concourse.bacc as bacc
import concourse.bass as bass
import concourse.mybir as mybir
import concourse.tile as tile
from concourse.bass_utils import run_bass_kernel_spmd

F32 = mybir.dt.float32
F32R = mybir.dt.float32r
BF16 = mybir.dt.bfloat16
I16 = mybir.dt.int16
AF = mybir.ActivationFunctionType
ALU = mybir.AluOpType

B, N, C = 4, 2048, 1024
H, D, R = 16, 64, 64
NT = 2048          # kv tokens per core (full batch sequence)
NQ = 1024          # query tokens per core
EPS = 1e-6
SCALE = D ** -0.5
N_CORES = 8

# Schraudolph exp in bf16 bit-space: bf16(bits = round(x*A + B)) ~= exp(x)
A_SCH = float((128.0 / np.log(2.0)) * SCALE)
B_SCH = float(127 * 128) - 7.35
# which exp tiles go to DVE (indices mod 13); 5/13 ~= 38% of tiles
DVE_EXP_SLOTS = frozenset((1, 4, 7, 10))

# bumped on every kernel revision; padded into ones2_t's shape so the HLO
# fingerprint changes and the terminal-side staged-executable cache cannot
# serve a stale NEFF for a prior revision
KERNEL_VERSION = 10


def build_nc(reps: int = 1, ablate=()):
    ablate = set(ablate)
    nc = bacc.Bacc("TRN2", target_bir_lowering=False)
    x_t = nc.dram_tensor("x_t", [C, NT], BF16, kind="ExternalInput")
    wa_t = nc.dram_tensor("wa_t", [C, 2 * R], BF16, kind="ExternalInput")
    m_t = nc.dram_tensor("m_t", [8 * 128, 128], BF16, kind="ExternalInput")
    wv_t = nc.dram_tensor("wv_t", [R, H * D], BF16, kind="ExternalInput")
    wp_t = nc.dram_tensor("wp_t", [H * D, C], BF16, kind="ExternalInput")
    ones2_t = nc.dram_tensor("ones2_t", [128, 2 + KERNEL_VERSION], F32R,
                             kind="ExternalInput")
    y_out = nc.dram_tensor("y_out", [NQ, C], F32, kind="ExternalOutput")
    out_r = y_out.rearrange("(qt p) c -> p qt c", p=128)

    exp_ctr = [0]

    with tile.TileContext(nc) as tc:
        with (
            tc.tile_pool(name="wsb", bufs=1) as wsb,
            tc.tile_pool(name="res", bufs=1) as res,
            tc.tile_pool(name="xs", bufs=4) as xs,
            tc.tile_pool(name="work", bufs=2) as work,
            tc.tile_pool(name="pts", bufs=4) as pts,
            tc.tile_pool(name="small", bufs=2) as small,
            tc.tile_pool(name="drp", bufs=2, space="DRAM") as drp,
        ):

            def bcast_ap(dram_row, n_part):
                return bass.AP(tensor=dram_row.tensor, offset=dram_row.offset,
                               ap=[[0, n_part]] + list(dram_row.ap[1:]))

            def emit_exp(pt, st):
                """P = exp(S*scale): ACT true exp or DVE Schraudolph."""
                slot = exp_ctr[0] % 13
                exp_ctr[0] += 1
                if slot in DVE_EXP_SLOTS and "noschraud" not in ablate:
                    if "splitdve" in ablate:
                        pt_i16 = pt.bitcast(I16)
                        nc.vector.tensor_scalar(pt_i16[:, 0:512], st[:, 0:512],
                                                A_SCH, B_SCH, ALU.mult, ALU.add)
                        nc.vector.tensor_scalar(pt_i16[:, 512:1024],
                                                st[:, 512:1024],
                                                A_SCH, B_SCH, ALU.mult, ALU.add)
                    else:
                        nc.vector.tensor_scalar(pt.bitcast(I16), st,
                                                A_SCH, B_SCH, ALU.mult, ALU.add)
                else:
                    nc.scalar.activation(pt, st, AF.Exp, scale=SCALE)

            import contextlib

            def loop_ctx():
                if reps > 1:
                    return tc.For_i(0, reps, 1)
                return contextlib.nullcontext()

            with loop_ctx():
                # ---- first x chunk + latent weights before bulk weights ----
                xt0 = xs.tile([128, 8, 512], BF16, tag="x", name="xt0")
                for kc in range(8):
                    nc.sync.dma_start(
                        xt0[:, kc, :], x_t[kc * 128:(kc + 1) * 128, 0:512])
                wa_sb = wsb.tile([128, 8, 2 * R], BF16, tag="wa")
                for kc in range(8):
                    nc.sync.dma_start(wa_sb[:, kc, :], wa_t[kc * 128:(kc + 1) * 128, :])
                m_sb = wsb.tile([128, 8, 128], BF16, tag="m")
                for hp in range(8):
                    nc.sync.dma_start(m_sb[:, hp, :],
                                      m_t[hp * 128:(hp + 1) * 128, :])
                wv_sb = wsb.tile([128, H * D], BF16, tag="wv")
                nc.sync.dma_start(wv_sb[0:64, :], wv_t[:])
                wp_sb = wsb.tile([128, 8, C], BF16, tag="wp")
                for hp in range(8):
                    nc.sync.dma_start(wp_sb[:, hp, :], wp_t[hp * 128:(hp + 1) * 128, :])

                # ---- constants ----
                ones_bf = wsb.tile([128, 1], BF16, tag="ones")
                nc.vector.memset(ones_bf[:], 1.0)
                eps2 = wsb.tile([2, 1], F32, tag="eps")
                nc.vector.memset(eps2[:], EPS)
                ones2 = wsb.tile([128, 2], F32R, tag="ones2")
                nc.sync.dma_start(ones2[:], ones2_t[:, 0:2])

                # ---- resident tensors ----
                lat_n = res.tile([128, NT], BF16, tag="lat_n")
                ybuf = res.tile([128, 8, NQ], BF16, tag="ybuf")

                # ---- phase 0: fused latents + rmsnorm ----
                with tc.tile_pool(name="ps0", bufs=2, space="PSUM") as ps0:
                    lat_ps = ps0.tile([128, NT], F32, tag="lat", bufs=1)
                    # stream by token chunk: each [1024C, 512tok] block of
                    # x arrives as 8 sub-DMAs; latent accumulation and
                    # rmsnorm for chunk t4 overlap chunk t4+1's DMA
                    for t4 in range(4):
                        if t4 == 0:
                            xt = xt0
                        else:
                            xt = xs.tile([128, 8, 512], BF16, tag="x")
                            for kc in range(8):
                                nc.sync.dma_start(
                                    xt[:, kc, :],
                                    x_t[kc * 128:(kc + 1) * 128,
                                        t4 * 512:(t4 + 1) * 512])
                        for kc in range(8):
                            nc.tensor.matmul(
                                lat_ps[:, t4 * 512:(t4 + 1) * 512],
                                wa_sb[:, kc, :],
                                xt[:, kc, :],
                                start=(kc == 0), stop=(kc == 7))
                    ssq4 = ps0.tile([2, 4, 512], F32, tag="aux", bufs=1)
                    for t4 in range(4):
                        sl = bass.ts(t4, 512)
                        sq = small.tile([128, 512], F32R, tag="sq")
                        nc.scalar.activation(sq[:], lat_ps[:, sl], AF.Square)
                        nc.tensor.matmul(ssq4[:, t4, :], ones2[:], sq[:],
                                         start=True, stop=True)
                    # batched rmsnorm tail: one Ln, one Exp (single ACT
                    # table-set load), one DRAM bounce for all 4 chunks
                    lns = small.tile([2, 4, 512], F32, tag="lns")
                    nc.scalar.activation(lns[:], ssq4[:], AF.Ln,
                                         bias=eps2[:], scale=1.0 / R)
                    inv = small.tile([2, 4, 512], F32R, tag="inv")
                    nc.scalar.activation(inv[:], lns[:], AF.Exp, scale=-0.5)
                    inv_d = drp.tile([2, 4, 512], F32R, tag="inv_d")
                    nc.sync.dma_start(inv_d[:], inv[:])
                    bc_sb = small.tile([128, 4, 512], F32R, tag="bc4")
                    nc.sync.dma_start(bc_sb[0:64, :, :],
                                      bcast_ap(inv_d[0:1, :, :], 64))
                    nc.sync.dma_start(bc_sb[64:128, :, :],
                                      bcast_ap(inv_d[1:2, :, :], 64))
                    for t4 in range(4):
                        sl = bass.ts(t4, 512)
                        nc.vector.tensor_mul(lat_n[:, sl], lat_ps[:, sl],
                                             bc_sb[:, t4, :])

                # ---- phase 1: head pairs ----
                if "p0only" in ablate:
                    for qt in range(4):
                        osb0 = small.tile([128, 512], F32, tag="osb")
                        nc.vector.tensor_copy(osb0[:], lat_n[:, bass.ts(qt, 512)])
                        nc.sync.dma_start(out_r[:, qt, 0:512], osb0[:])
                    nc.compile()
                    return nc
                # duplicate kv-latent at partitions 64-127 (row-group pairing)
                lat_kv2 = res.tile([128, NT], BF16, tag="lat_kv2")
                nc.sync.dma_start(lat_kv2[64:128, :], lat_n[0:64, :])

                with (
                    tc.tile_pool(name="pst", bufs=3, space="PSUM") as pst,
                    tc.tile_pool(name="psy", bufs=2, space="PSUM") as psy,
                ):
                    def kqv_tiles_and_thunks(hp):
                        """Allocate K.T/Q.T/V tiles for pair hp and return a
                        list of emission thunks (matmul+evac units)."""
                        hsl = bass.ts(hp, 128)
                        zpr = work.tile([128, NQ], BF16, tag="zpr",
                                        name=f"zpr{hp}")
                        vt = work.tile([128, 16, 130], BF16, tag="vt",
                                       name=f"vt{hp}")
                        vt2 = vt.rearrange("p k (s u) -> p k s u", s=2)
                        thunks = []

                        def z_unit(t2):
                            # Z_pair = [M_A | M_B]^T @ L_q: one M=128 matmul
                            # produces both heads' Z (rows 0-63 A, 64-127 B)
                            sl = bass.ts(t2, 512)
                            zps = pst.tile([128, 1024], F32, tag="st",
                                           name="zps")[:, 0:512]
                            nc.tensor.matmul(zps[:],
                                             m_sb[64:128, hp, :],
                                             lat_n[64:128, sl],
                                             start=True, stop=True)
                            nc.vector.tensor_copy(zpr[:, sl], zps[:])

                        def v_unit(kt0):
                            if "basev" in ablate:
                                for kt in (kt0, kt0 + 1, kt0 + 2, kt0 + 3):
                                    vps = pst.tile([128, 1024], F32, tag="st",
                                                   name="vps1")[:, 0:128]
                                    nc.tensor.matmul(
                                        vps[:], lat_n[0:64, bass.ts(kt, 128)],
                                        wv_sb[0:64, hsl], start=True, stop=True)
                                    nc.vector.tensor_copy(
                                        vt2[:, kt, :, 0:64],
                                        vps[:].rearrange("p (s u) -> p s u", s=2))
                                return
                            # four V k-tiles into one psum bank, all on the
                            # SAME row group so the matmuls serialize -- two
                            # row-group-concurrent matmuls draining into one
                            # single-port psum bank is a fatal HW collision.
                            # Single strided DVE evac for all four.
                            vps = pst.tile([128, 1024], F32, tag="st",
                                           name="vps")[:, 0:512]
                            for j, kt in enumerate(range(kt0, kt0 + 4)):
                                vsl = bass.ts(j, 128)
                                nc.tensor.matmul(
                                    vps[:, vsl],
                                    lat_n[0:64, bass.ts(kt, 128)],
                                    wv_sb[0:64, hsl], start=True, stop=True)
                            nc.vector.tensor_copy(
                                vt2[:, kt0:kt0 + 4, :, 0:64],
                                vps[:].rearrange("p (k s u) -> p k s u",
                                                 k=4, s=2))

                        def ones_unit():
                            nc.vector.tensor_copy(
                                vt2[:, :, :, 64:65],
                                ones_bf[:].broadcast_to([128, 16, 2, 1]))

                        for t2 in range(2):
                            thunks.append(lambda t2=t2: z_unit(t2))
                        thunks.append(ones_unit)
                        for kt0 in range(0, 16, 4):
                            thunks.append(lambda kt0=kt0: v_unit(kt0))
                        return (zpr, vt), thunks

                    NHP = 1 if "onehp" in ablate else 8
                    norm_q = []
                    cur_tiles, cur_thunks = kqv_tiles_and_thunks(0)
                    for th in cur_thunks:
                        th()
                    pending = []
                    for hp in range(NHP):
                        zpr, vt = cur_tiles
                        if hp < NHP - 1:
                            cur_tiles, pending = kqv_tiles_and_thunks(hp + 1)
                        else:
                            pending = []
                        # attention: per qc a single chain; per kt one
                        # [128,1024] (A|B) psum group -> one exp; y matmuls
                        # delayed one kt (PE FIFO never blocks on exp).
                        for qc in range(2):
                            qsl = bass.ts(qc, 512)
                            ya = psy.tile([65, 512], F32, tag="y",
                                          name=f"ya{hp}_{qc}")
                            yb = psy.tile([65, 512], F32, tag="y",
                                          name=f"yb{hp}_{qc}")

                            def emit_y(kt, pt, ya=ya, yb=yb, vt=vt):
                                nc.tensor.matmul(ya[:], vt[:, kt, 0:65],
                                                 pt[:, 0:512],
                                                 start=(kt == 0), stop=(kt == 15))
                                nc.tensor.matmul(yb[:], vt[:, kt, 65:130],
                                                 pt[:, 512:1024],
                                                 start=(kt == 0), stop=(kt == 15))

                            from collections import deque
                            pipe = deque()
                            DEPTH = 2
                            for kt in range(16):
                                ksl = bass.ts(kt, 128)
                                st = pst.tile([128, 1024], F32, tag="st")
                                nc.tensor.matmul(st[:, 0:512],
                                                 lat_n[0:64, ksl],
                                                 zpr[0:64, qsl],
                                                 start=True, stop=True)
                                nc.tensor.matmul(st[:, 512:1024],
                                                 lat_kv2[64:128, ksl],
                                                 zpr[64:128, qsl],
                                                 start=True, stop=True)
                                pt = pts.tile([128, 1024], BF16, tag="pt")
                                emit_exp(pt[:], st[:])
                                pipe.append((kt, pt))
                                if norm_q:
                                    if kt == 0:
                                        norm_state = [norm_q[0][0]()]
                                    elif kt == 1:
                                        norm_state.append(
                                            norm_q[0][1](norm_state[0]))
                                    elif kt == 4:
                                        s1, s2, s3 = norm_q.pop(0)
                                        s3(norm_state[0], norm_state[1])
                                if len(pipe) > DEPTH:
                                    emit_y(*pipe.popleft())
                                if pending:
                                    pending.pop(0)()
                            while pipe:
                                emit_y(*pipe.popleft())
                            # normalize, staged into the NEXT iteration's
                            # kt loop: (1) evacuate ya/yb to SBUF asap so the
                            # psum bank frees before next iteration's y
                            # matmuls need it; (2) reciprocals + DRAM-bounce
                            # broadcast; (3) muls from the SBUF copies. This
                            # keeps both the PE (psum slots) and the DVE
                            # FIFO (bounce latency) off the critical path.
                            def stage1(ya=ya, yb=yb):
                                ysb = small.tile([65, 2, 512], F32, tag="ysb")
                                nc.vector.tensor_copy(ysb[:, 0, :], ya[:])
                                nc.vector.tensor_copy(ysb[:, 1, :], yb[:])
                                return ysb

                            def stage2(ysb):
                                rq = small.tile([1, 2, 512], F32R, tag="rq")
                                with nc.allow_low_precision(
                                        reason="f32r softmax denominators"):
                                    nc.vector.reciprocal(rq[0:1, 0, :],
                                                         ysb[64:65, 0, :])
                                    nc.vector.reciprocal(rq[0:1, 1, :],
                                                         ysb[64:65, 1, :])
                                rq_d = drp.tile([1, 2, 512], F32R, tag="rq_d")
                                nc.sync.dma_start(rq_d[:], rq[:])
                                bcy = small.tile([64, 2, 512], F32R, tag="bcy")
                                nc.sync.dma_start(bcy[:, 0, :],
                                                  bcast_ap(rq_d[0:1, 0, :], 64))
                                nc.sync.dma_start(bcy[:, 1, :],
                                                  bcast_ap(rq_d[0:1, 1, :], 64))
                                return bcy

                            def stage3(ysb, bcy, hp=hp, qsl=qsl):
                                nc.vector.tensor_mul(ybuf[0:64, hp, qsl],
                                                     ysb[0:64, 0, :],
                                                     bcy[:, 0, :])
                                y2b = small.tile([64, 512], BF16, tag="y2b")
                                nc.vector.tensor_mul(y2b[:], ysb[0:64, 1, :],
                                                     bcy[:, 1, :])
                                nc.sync.dma_start(ybuf[64:128, hp, qsl],
                                                  y2b[:])
                            norm_q.append((stage1, stage2, stage3))
                        for th in pending:
                            th()
                    while norm_q:
                        s1, s2, s3 = norm_q.pop(0)
                        ysb_f = s1()
                        bcy_f = s2(ysb_f)
                        s3(ysb_f, bcy_f)
                    # ---- proj ----
                    if "noproj" in ablate:
                        nc.sync.dma_start(out_r[:, :, 0:512], ybuf[:].bitcast(F32))
                        nc.compile()
                        return nc
                    for qt in range(8):
                        for cc in range(2):
                            pj = pst.tile([128, 1024], F32, tag="st",
                                          name="pj")[:, 0:512]
                            for hp2 in range(8):
                                nc.tensor.matmul(
                                    pj[:], ybuf[:, hp2, bass.ts(qt, 128)],
                                    wp_sb[:, hp2, bass.ts(cc, 512)],
                                    start=(hp2 == 0), stop=(hp2 == 7))
                            osb = small.tile([128, 512], F32, tag="osb")
                            nc.vector.tensor_copy(osb[:], pj[:])
                            nc.sync.dma_start(out_r[:, qt, bass.ts(cc, 512)],
                                              osb[:])
    nc.compile()
    return nc


BF = ml_dtypes.bfloat16


def prep_inputs(x, w_kv_a, w_kv_b, w_q_a, w_q_b, w_proj, kv_norm_w, q_norm_w):
    """Host-side sharding/layout prep. Returns per-core input maps."""
    x = np.asarray(x, dtype=np.float32)
    w_kv_b = np.asarray(w_kv_b, dtype=np.float32) * np.asarray(kv_norm_w, np.float32)[None, :]
    w_q_b = np.asarray(w_q_b, dtype=np.float32) * np.asarray(q_norm_w, np.float32)[None, :]
    wa_t = np.ascontiguousarray(
        np.concatenate([np.asarray(w_kv_a, np.float32),
                        np.asarray(w_q_a, np.float32)], axis=0).T).astype(BF)
    kvb = w_kv_b.reshape(H, 2, D, R)
    wv_t = np.ascontiguousarray(
        kvb[:, 1].transpose(2, 0, 1).reshape(R, H * D)).astype(BF)
    # per-head folded S-matrix M_h = Wq_h^T @ Wk_h  [R, R]
    m_t = np.zeros((8 * 128, 128), np.float32)
    for hp_ in range(8):
        wq_a_h = w_q_b[(2 * hp_) * D:(2 * hp_ + 1) * D, :]        # [D, R]
        wq_b_h = w_q_b[(2 * hp_ + 1) * D:(2 * hp_ + 2) * D, :]
        wk_a_h = kvb[2 * hp_, 0]                                  # [D, R]
        wk_b_h = kvb[2 * hp_ + 1, 0]
        m_t[hp_ * 128 + 64:(hp_ + 1) * 128, 0:64] = wq_a_h.T @ wk_a_h
        m_t[hp_ * 128 + 64:(hp_ + 1) * 128, 64:128] = wq_b_h.T @ wk_b_h
    m_t = m_t.astype(BF)
    wp_t = np.ascontiguousarray(np.asarray(w_proj, np.float32).T).astype(BF)

    in_maps = []
    for core in range(N_CORES):
        b, half = divmod(core, 2)
        own = x[b, half * NQ:(half + 1) * NQ]
        other = x[b, (1 - half) * NQ:(2 - half) * NQ]
        x_perm_t = np.ascontiguousarray(
            np.concatenate([own, other], axis=0).T).astype(BF)
        in_maps.append({
            "x_t": x_perm_t, "wa_t": wa_t, "m_t": m_t,
            "wv_t": wv_t, "wp_t": wp_t, "ones2_t": _ONES2,
        })
    return in_maps


def assemble_output(results):
    out = np.empty((B, N, C), dtype=np.float32)
    for core in range(N_CORES):
        b, half = divmod(core, 2)
        out[b, half * NQ:(half + 1) * NQ] = results[core]["y_out"]
    return out


_ONES2 = np.zeros((128, 2 + KERNEL_VERSION), np.float32)
_ONES2[0:64, 0] = 1.0
_ONES2[64:128, 1] = 1.0

_NC_CACHE = {}


def kernel(**inputs) -> np.ndarray:
    if 1 not in _NC_CACHE:
        _NC_CACHE[1] = build_nc(reps=1)
    nc = _NC_CACHE[1]
    in_maps = prep_inputs(**inputs)
    res = run_bass_kernel_spmd(nc, in_maps, core_ids=list(range(N_CORES)))
    return assemble_output(res.results)
